# revision 1
# baseline (speedup 1.0000x reference)
"""HeadFusionAttention Trainium2 kernel (8 NeuronCores, data-parallel over B).

Reference computation (per batch b):
    head_x = 0
    for i in 0..3:                                  # sequential group chain
        cur   = x[:, 256*i:256*(i+1)] + head_x      # [N, 256]
        qkv   = cur @ qkv_w[i].T                    # [N, 768] -> q,k,v [N,256]
        S     = (q @ k.T) * SCALE                   # [N, N]
        P     = softmax(S, axis=-1)
        head_x = P @ v                              # [N, 256]
        y[:, 256*i:256*(i+1)] = head_x
    out = y @ proj_w.T + proj_b                     # [N, 1024]

Sharding: B=16 batches split 2 per core across 8 cores; weights replicated;
zero collectives. All activations are kept transposed on-chip ([feature, n]
layout) so every matmul consumes operands in natural [K, M]/[K, N] form and
no on-chip transposes are needed:
  - qkv^T = wT.T-matmul(cur^T)            (q^T, k^T in [d, n])
  - v is produced directly in [n, d] via swapping matmul operands
  - S^T   = k^T-matmul(q^T)  ([k_n, q_n]); softmax runs along partitions:
    exp via ACT (scale folded in, no max subtraction -- scores are O(1)),
    denominators via ones-vector matmul, broadcast via stride-0-partition DMA
  - head_x^T = v-matmul(P^T), normalized by reciprocal denominators
  - out^T accumulates per-group projection partials (bias folded into the
    first partial); host transposes the [e, n] result back.
"""

import numpy as np

B, N, DIM = 16, 1024, 1024
G = 4
G_DIM = 256
SCALE = 128 ** -0.5
N_CORES = 8
B_PER = B // N_CORES  # 2

P = 128          # SBUF partitions
FH = 512         # free-dim half (psum bank: 512 fp32)
USE_F32R = True  # fp32r matmuls: full-rate single pass (vs 4x slower fp32)


def build_nc(use_f32r=USE_F32R):
    from contextlib import ExitStack

    import concourse.mybir as mybir
    import concourse.tile as tile
    from concourse import bacc

    f32 = mybir.dt.float32
    # float32r: same 4-byte layout as fp32 but single-pass full-rate matmul.
    # The BIR verifier requires every matmul operand's producer to emit
    # float32r, so all matmul-feeding tiles/params are typed float32r.
    mdt = mybir.dt.float32r if use_f32r else f32

    # Bacc (vs plain Bass) runs the wait-splitting passes walrus requires
    nc = bacc.Bacc()

    xT = nc.declare_dram_parameter("xT", [B_PER, DIM, N], mdt, isOutput=False)
    wqkvT = nc.declare_dram_parameter("wqkvT", [G, G_DIM, 3 * G_DIM], mdt, isOutput=False)
    pwT = nc.declare_dram_parameter("pwT", [DIM, DIM], mdt, isOutput=False)
    pb = nc.declare_dram_parameter("pb", [P, DIM // P], f32, isOutput=False)
    outT = nc.declare_dram_parameter("outT", [B_PER, DIM, N], f32, isOutput=True)

    Exp = mybir.ActivationFunctionType.Exp
    Ident = mybir.ActivationFunctionType.Identity

    with tile.TileContext(nc) as tc, ExitStack() as ctx:
        consts = ctx.enter_context(tc.tile_pool(name="consts", bufs=1))
        pw_pool = ctx.enter_context(tc.tile_pool(name="pw_pool", bufs=2))
        acc_pool = ctx.enter_context(tc.tile_pool(name="acc_pool", bufs=1))
        cur_pool = ctx.enter_context(tc.tile_pool(name="cur_pool", bufs=3))
        qk_pool = ctx.enter_context(tc.tile_pool(name="qk_pool", bufs=1))
        v_pool = ctx.enter_context(tc.tile_pool(name="v_pool", bufs=2))
        pt_pool = ctx.enter_context(tc.tile_pool(name="pt_pool", bufs=4))
        hx_pool = ctx.enter_context(tc.tile_pool(name="hx_pool", bufs=3))
        sm_pool = ctx.enter_context(tc.tile_pool(name="sm_pool", bufs=2))

        ps_mm = ctx.enter_context(tc.tile_pool(name="ps_mm", bufs=2, space="PSUM"))
        ps_s = ctx.enter_context(tc.tile_pool(name="ps_s", bufs=2, space="PSUM"))
        ps_pv = ctx.enter_context(tc.tile_pool(name="ps_pv", bufs=3, space="PSUM"))
        ps_den = ctx.enter_context(tc.tile_pool(name="ps_den", bufs=1, space="PSUM"))

        # ---- constants ----
        # qkv weights, transposed: [d partition, group, d-subtile, e]
        wq_sb = consts.tile([P, G, 2, 3 * G_DIM], mdt)
        nc.sync.dma_start(
            out=wq_sb,
            in_=wqkvT.rearrange("g (ds p) e -> p g ds e", p=P),
        )
        pb_sb = consts.tile([P, DIM // P], f32)
        nc.sync.dma_start(out=pb_sb, in_=pb[:, :])
        # memset can't write float32r directly; stage via f32 + copy
        ones_f32 = consts.tile([P, P], f32)
        nc.vector.memset(ones_f32, 1.0)
        ones_col = consts.tile([P, 1], mdt)
        nc.vector.tensor_copy(ones_col, ones_f32[:, 0:1])
        ones_sq = consts.tile([P, P], mdt)
        nc.vector.tensor_copy(ones_sq, ones_f32)
        zero_f32 = consts.tile([P, FH], f32)
        nc.vector.memset(zero_f32, 0.0)
        # reciprocal staging tiles: row 0 carries data, rows 1.. stay zero so
        # an all-ones matmul broadcasts row 0 across all partitions
        rec_pads = []
        rec_rows = []
        for hh in range(2):
            rec_pad = consts.tile([P, FH], mdt, name=f"rec_pad{hh}")
            nc.vector.tensor_copy(rec_pad, zero_f32)
            rec_pads.append(rec_pad)
            rec_row = consts.tile([1, FH], f32, name=f"rec_row{hh}")
            rec_rows.append(rec_row)

        for b in range(B_PER):
            out_acc = acc_pool.tile([P, DIM // P, N], f32)

            # first group's input: x^T rows 0:256
            cur = cur_pool.tile([P, 2, N], mdt, tag="cur")
            nc.sync.dma_start(
                out=cur, in_=xT[b, 0:G_DIM].rearrange("(ds p) n -> p ds n", p=P)
            )

            prev_hx = None  # (hx_tile, group_idx) pending projection partial
            for i in range(G):
                # projection weight slice for this group, prefetched
                pw_s = pw_pool.tile([P, 2, DIM], mdt)
                nc.sync.dma_start(
                    out=pw_s,
                    in_=pwT[G_DIM * i : G_DIM * (i + 1)].rearrange(
                        "(ds p) e -> p ds e", p=P
                    ),
                )

                # ---- A: q^T, k^T [e-chunk, n] = w_qk.T-matmul(cur^T) ----
                qkT = qk_pool.tile([P, 4, N], mdt)
                for ec in range(4):
                    for h in range(2):
                        ps = ps_mm.tile([P, FH], f32, tag="ps_mm")
                        for ds in range(2):
                            nc.tensor.matmul(
                                ps,
                                (wq_sb[:, i, ds, P * ec : P * (ec + 1)]),
                                (cur[:, ds, FH * h : FH * (h + 1)]),
                                start=(ds == 0),
                                stop=(ds == 1),
                            )
                        nc.vector.tensor_copy(qkT[:, ec, FH * h : FH * (h + 1)], ps)

                # ---- B: v [n-chunk, d] = cur-matmul(w_v) ----
                v_sb = v_pool.tile([P, 8, G_DIM], mdt)
                for nk in range(8):
                    ps = ps_mm.tile([P, FH], f32, tag="ps_mm")
                    for ds in range(2):
                        nc.tensor.matmul(
                            ps[:, :G_DIM],
                            (cur[:, ds, P * nk : P * (nk + 1)]),
                            (wq_sb[:, i, ds, 2 * G_DIM : 3 * G_DIM]),
                            start=(ds == 0),
                            stop=(ds == 1),
                        )
                    nc.vector.tensor_copy(v_sb[:, nk], ps[:, :G_DIM])

                # next group's x slice (overwritten into cur_next, then += hx)
                cur_next = None
                if i + 1 < G:
                    cur_next = cur_pool.tile([P, 2, N], mdt, tag="cur")
                    nc.sync.dma_start(
                        out=cur_next,
                        in_=xT[b, G_DIM * (i + 1) : G_DIM * (i + 2)].rearrange(
                            "(ds p) n -> p ds n", p=P
                        ),
                    )

                hx = hx_pool.tile([P, 2, N], mdt)

                # ---- attention, one q-half at a time ----
                for h in range(2):
                    pv_ps = [
                        ps_pv.tile([P, FH], f32, tag="ps_pv", name=f"pv_ps{dc}")
                        for dc in range(2)
                    ]
                    den_ps = ps_den.tile([1, FH], f32, tag="ps_den")
                    for kc in range(8):
                        s_ps = ps_s.tile([P, FH], f32, tag="ps_s")
                        for ds in range(2):
                            nc.tensor.matmul(
                                s_ps,
                                (qkT[:, 2 + ds, P * kc : P * (kc + 1)]),
                                (qkT[:, ds, FH * h : FH * (h + 1)]),
                                start=(ds == 0),
                                stop=(ds == 1),
                            )
                        pt = pt_pool.tile([P, FH], mdt)
                        nc.scalar.activation(pt, s_ps, Exp, scale=SCALE)
                        nc.tensor.matmul(
                            den_ps,
                            (ones_col),
                            (pt),
                            start=(kc == 0),
                            stop=(kc == 7),
                        )
                        for dc in range(2):
                            nc.tensor.matmul(
                                pv_ps[dc],
                                (v_sb[:, kc, P * dc : P * (dc + 1)]),
                                (pt),
                                start=(kc == 0),
                                stop=(kc == 7),
                            )

                    # denominators -> reciprocal on row 0, broadcast across
                    # partitions with an all-ones matmul (rows 1.. are zero)
                    nc.vector.reciprocal(rec_rows[h], den_ps)
                    with nc.allow_low_precision(reason="fp32r bcast staging"):
                        nc.vector.tensor_copy(rec_pads[h][0:1, :], rec_rows[h])
                    bc_ps = ps_mm.tile([P, FH], f32, tag="ps_mm")
                    nc.tensor.matmul(
                        bc_ps, (ones_sq), (rec_pads[h]),
                        start=True, stop=True,
                    )
                    rec_b = sm_pool.tile([P, FH], f32, tag="rec_b")
                    nc.vector.tensor_copy(rec_b, bc_ps)

                    for dc in range(2):
                        nc.vector.tensor_mul(
                            hx[:, dc, FH * h : FH * (h + 1)], pv_ps[dc], rec_b
                        )
                        if cur_next is not None:
                            nc.vector.tensor_add(
                                cur_next[:, dc, FH * h : FH * (h + 1)],
                                cur_next[:, dc, FH * h : FH * (h + 1)],
                                hx[:, dc, FH * h : FH * (h + 1)],
                            )

                # ---- projection partial of the PREVIOUS group (fills PE
                # during this group's softmax tail) ----
                for hx_done, gi, pw_done in ([(prev_hx[0], prev_hx[1], prev_hx[2])] if prev_hx else []):
                    _emit_proj_partial(nc, ps_mm, out_acc, hx_done, gi, pw_done, pb_sb, f32, Ident)
                prev_hx = (hx, i, pw_s)
                cur = cur_next

            # last group's projection partial, then store out^T
            _emit_proj_partial(nc, ps_mm, out_acc, prev_hx[0], prev_hx[1], prev_hx[2], pb_sb, f32, Ident)
            for ec in range(DIM // P):
                nc.sync.dma_start(
                    out=outT[b, P * ec : P * (ec + 1)], in_=out_acc[:, ec]
                )

    nc.finalize()
    return nc


def _emit_proj_partial(nc, ps_mm, out_acc, hx, gi, pw_s, pb_sb, f32, Ident):
    """out_acc[:, ec, :] (+)= pw_s.T-matmul(hx);  group 0 also adds bias."""
    for ec in range(DIM // P):
        for h in range(2):
            ps = ps_mm.tile([P, FH], f32, tag="ps_mm")
            for ds in range(2):
                nc.tensor.matmul(
                    ps,
                    (pw_s[:, ds, P * ec : P * (ec + 1)]),
                    (hx[:, ds, FH * h : FH * (h + 1)]),
                    start=(ds == 0),
                    stop=(ds == 1),
                )
            dst = out_acc[:, ec, FH * h : FH * (h + 1)]
            if gi == 0:
                nc.scalar.activation(dst, ps, Ident, bias=pb_sb[:, ec : ec + 1])
            else:
                nc.vector.tensor_add(dst, dst, ps)


def _host_prep(x, qkv_w, proj_w, proj_b):
    xT = np.ascontiguousarray(x.transpose(0, 2, 1))              # [B, DIM, N]
    wqkvT = np.ascontiguousarray(qkv_w.transpose(0, 2, 1))       # [G, 256, 768]
    pwT = np.ascontiguousarray(proj_w.T)                         # [DIM, DIM]
    pb = np.ascontiguousarray(proj_b.reshape(DIM // P, P).T)     # [128, 8]
    return xT, wqkvT, pwT, pb


def kernel(x, qkv_w, proj_w, proj_b):
    from concourse.bass_utils import run_bass_kernel_spmd

    x = np.asarray(x, dtype=np.float32)
    qkv_w = np.asarray(qkv_w, dtype=np.float32)
    proj_w = np.asarray(proj_w, dtype=np.float32)
    proj_b = np.asarray(proj_b, dtype=np.float32)

    xT, wqkvT, pwT, pb = _host_prep(x, qkv_w, proj_w, proj_b)

    nc = build_nc()
    in_maps = [
        {
            "xT": np.ascontiguousarray(xT[c * B_PER : (c + 1) * B_PER]),
            "wqkvT": wqkvT,
            "pwT": pwT,
            "pb": pb,
        }
        for c in range(N_CORES)
    ]
    res = run_bass_kernel_spmd(nc, in_maps, core_ids=list(range(N_CORES)))
    shards = [res.results[c]["outT"] for c in range(N_CORES)]
    outT = np.concatenate(shards, axis=0)          # [B, DIM, N]
    return np.ascontiguousarray(outT.transpose(0, 2, 1)).astype(np.float32)


if __name__ == "__main__":
    import sys

    if len(sys.argv) > 1 and sys.argv[1] == "build":
        nc = build_nc()
        print("build OK, instructions:", sum(1 for _ in nc.m.functions[0].instructions)
              if hasattr(nc.m.functions[0], "instructions") else "?")

